# revision 17
# baseline (speedup 1.0000x reference)
"""BlockSparseLinear hybrid fp8/bf16 kernel for Trainium2 (8 NeuronCores).

y = x @ W.T + bias, x [8192,4096] f32, W [4096,4096] f32 (50% of 16x16
blocks zeroed), bias [4096]; 8-way data-parallel over tokens.

v3 over the 411us baseline (v2 lost its PE-work win to a DMA-bound
head, measured 417us):
- fp8/bf16 split chosen per (out-tile, k-tile) 128x128 tile: greedy
  pick of lightest-noise tiles fits ~324 fp8 tiles in the 2e-2 budget
  vs 256 for the per-k split (predicted 1.97e-2). fp8 pairs run as
  DoubleRow matmuls with stepped rhs APs; weights packed per-out-tile.
- The 8MB bf16 x copy is NOT DMA'd: it is reconstructed on the idle
  DVE as bf16(x8 + xr8) from the fp8 x and an fp8 residual stream
  (4MB), cutting input x traffic from 12MB to 8MB. Reconstruction is
  as accurate as direct bf16 quantization.
- x streams are packed window-major in DRAM so every transfer is
  contiguous per partition (v2's 512B strided runs tanked DMA
  efficiency).
- Startup: the 7 fp8-heaviest out-tiles' window-0 DR groups are
  hoisted to the front (PSUM banks 1-7) covering the x-residual
  stream; phase 1 closes them with bf16, phase 2 runs their window 1,
  phase 3 the rest in ascending fp8-count order (cheapest tail last).
  Input DMAs issue from sync in priority order; wb streams from scalar
  paced one per eviction; w8 prefetch deferred to phase 2.
- Steady windows run bf16 first, fp8 pairs last.
- Last out-tile's final window: two 256-wide quarters,
  partition-split stores.
"""

import os

import numpy as np

N_CORES = 8
TOK = 8192
T_PER_CORE = TOK // N_CORES  # 1024
D_IN = 4096
D_OUT = 4096
P = 128
KO = D_IN // P  # 32
OC = D_OUT // P  # 32
NT = 2
T_FREE = 512
SX = 32.0
SW = 1024.0
S_OUT = 1.0 / (SX * SW)
ERR_TARGET = 0.0199
HOIST = 7
WARM = 16
XR_CHUNK = 4  # max k-tiles per x-residual DMA/DVE-add chunk

LAST_EXEC_NS = None

_cache = {}


def _quant_arrays(x):
    import ml_dtypes

    E4 = ml_dtypes.float8_e4m3
    BF = ml_dtypes.bfloat16
    xs = x * SX
    x8 = xs.astype(E4)
    x8d = x8.astype(np.float32)
    xr8 = (xs - x8d).astype(E4)
    recon = (x8d + xr8.astype(np.float32)).astype(BF)
    return x8, xr8, x8d / SX, recon.astype(np.float32) / SX


def _select_tiles(x, w, x8d, xbd):
    """Greedy per-(oc,k) fp8 tile selection under the global error budget.

    Error model: independent per-element quantization noise,
    err^2(tile) = sum_{j,i in tile} (wq-w)_ij^2 * ||xq_i||^2
                                   + w_ij^2 * ||(xq-x)_i||^2,
    validated against exact simulation to ~3e-5 rel err. Budget is
    relative to ||y||^2 ~= sum_i ||x_i||^2 ||w_:i||^2 (0.9996 exact).
    xbd is the DVE-reconstructed bf16 x (dequantized). Returns sel
    [OC,KO] bool with even per-oc counts.
    """
    import heapq

    import ml_dtypes

    E4 = ml_dtypes.float8_e4m3
    BF = ml_dtypes.bfloat16

    w8d = ((w * SW).astype(E4).astype(np.float32)) / SW
    wbd = ((w * SW).astype(BF).astype(np.float32)) / SW

    S_x8 = (x8d**2).sum(0)
    S_xr8 = ((x8d - x) ** 2).sum(0)
    S_xb = (xbd**2).sum(0)
    S_xrb = ((xbd - x) ** 2).sum(0)
    W2 = w**2
    E8 = ((w8d - w) ** 2 * S_x8[None, :] + W2 * S_xr8[None, :]).reshape(
        OC, P, KO, P
    ).sum(axis=(1, 3))
    Eb = ((wbd - w) ** 2 * S_xb[None, :] + W2 * S_xrb[None, :]).reshape(
        OC, P, KO, P
    ).sum(axis=(1, 3))

    den = float((x**2).sum(0) @ W2.sum(0))
    budget = ERR_TARGET**2 * den - float(Eb.sum())
    delta = E8 - Eb
    order = np.argsort(delta, axis=1)

    sel = np.zeros((OC, KO), bool)
    heap = []
    for oc in range(OC):
        c = float(delta[oc, order[oc, 0]] + delta[oc, order[oc, 1]])
        heapq.heappush(heap, (c, oc, 0))
    spent = 0.0
    while heap:
        cost, oc, idx = heapq.heappop(heap)
        if spent + cost > budget:
            break
        spent += cost
        sel[oc, order[oc, idx]] = True
        sel[oc, order[oc, idx + 1]] = True
        if idx + 3 < KO:
            c = float(delta[oc, order[oc, idx + 2]] + delta[oc, order[oc, idx + 3]])
            heapq.heappush(heap, (c, oc, idx + 2))
    return sel


def _make_plan(sel):
    ns = sel.sum(axis=1).astype(int)  # [OC], even counts
    ks8 = [np.flatnonzero(sel[oc]).tolist() for oc in range(OC)]
    ksb = [np.flatnonzero(~sel[oc]).tolist() for oc in range(OC)]
    o8 = np.concatenate([[0], np.cumsum(ns)[:-1]]).astype(int).tolist()
    ob = np.concatenate([[0], np.cumsum(KO - ns)[:-1]]).astype(int).tolist()
    # hoisted: 7 largest fp8 counts (descending); rest ascending so the
    # final out-tile processed has the least bf16 work (short tail)
    desc = np.argsort(-ns, kind="stable")
    perm = desc[:HOIST].tolist() + desc[HOIST:][::-1].tolist()
    return {
        "ns": ns.tolist(),
        "ks8": ks8,
        "ksb": ksb,
        "o8": o8,
        "ob": ob,
        "perm": perm,
        "n8t": int(ns.sum()),
        "nbt": int((KO - ns).sum()),
        "n8max": int(ns.max()),
        "nbmax": int((KO - ns).max()),
    }


def _chunk_runs(ks_list, gap, chunk):
    """Bridged runs (merge gaps <= gap) split into <= chunk-sized pieces."""
    runs = []
    for k in sorted(ks_list):
        if runs and k - runs[-1][1] <= gap:
            runs[-1][1] = k + 1
        else:
            runs.append([k, k + 1])
    out = []
    for a, b in runs:
        while b - a > chunk:
            out.append((a, a + chunk))
            a += chunk
        out.append((a, b))
    return out


def _build_bass(plan):
    import concourse.bacc as bacc
    import concourse.mybir as mybir
    import concourse.tile as tile

    f32 = mybir.dt.float32
    f8 = mybir.dt.float8e4
    bf16 = mybir.dt.bfloat16
    DR = mybir.MatmulPerfMode.DoubleRow

    ns, ks8, ksb = plan["ns"], plan["ks8"], plan["ksb"]
    o8, ob, perm = plan["o8"], plan["ob"], plan["perm"]
    n8max, nbmax = plan["n8max"], plan["nbmax"]

    nc = bacc.Bacc(
        "TRN2",
        target_bir_lowering=False,
        debug=False,
        num_devices=N_CORES,
        name="block_sparse_linear_v3",
        dynamic_dma_scratch_size=4096,
    )

    xt8 = nc.dram_tensor("xt8", [NT, P, KO, T_FREE], f8, kind="ExternalInput")
    xtr = nc.dram_tensor("xtr", [NT, P, KO, T_FREE], f8, kind="ExternalInput")
    z0 = nc.dram_tensor("z0", [P, 2, T_FREE], f8, kind="ExternalInput")
    wp8 = nc.dram_tensor("wp8", [P, plan["n8t"], P], f8, kind="ExternalInput")
    wpb = nc.dram_tensor("wpb", [P, plan["nbt"], P], bf16, kind="ExternalInput")
    bs = nc.dram_tensor("bs", [P, OC], f32, kind="ExternalInput")
    yt = nc.dram_tensor("yt", [OC, P, T_PER_CORE], f32, kind="ExternalOutput")

    with tile.TileContext(nc) as tc:
        with (
            tc.tile_pool(name="xpool", bufs=1) as xpool,
            tc.tile_pool(name="xrpool", bufs=4) as xrpool,
            tc.tile_pool(name="w8pool", bufs=8) as w8pool,
            tc.tile_pool(name="wbpool", bufs=8) as wbpool,
            tc.tile_pool(name="opool", bufs=3) as opool,
            tc.tile_pool(name="pspool", bufs=8, space="PSUM") as pspool,
        ):
            x8_sb = xpool.tile([P, NT, KO, T_FREE], f8)
            xb_sb = xpool.tile([P, NT, KO, T_FREE], bf16)
            bias_sb = xpool.tile([P, OC], f32)

            w8_tiles = {}
            wb_tiles = {}

            def w8_dma(i):
                if i >= OC:
                    return
                n = ns[perm[i]]
                if not n:
                    return
                t = w8pool.tile([P, n8max, P], f8, tag="w8", name=f"w8_{i}")
                nc.sync.dma_start(
                    t[:, :n, :], wp8[:, o8[perm[i]] : o8[perm[i]] + n, :]
                )
                w8_tiles[i] = t

            def wb_dma(i):
                if i >= OC:
                    return
                m = KO - ns[perm[i]]
                if not m:
                    return
                t = wbpool.tile([P, nbmax, P], bf16, tag="wb", name=f"wb_{i}")
                h = (m + 1) // 2
                nc.scalar.dma_start(
                    t[:, :h, :], wpb[:, ob[perm[i]] : ob[perm[i]] + h, :]
                )
                nc.scalar.dma_start(
                    t[:, h:m, :], wpb[:, ob[perm[i]] + h : ob[perm[i]] + m, :]
                )
                wb_tiles[i] = t

            wb_cursor = [2]
            w8_cursor = [HOIST]
            w8_gate = [False]

            def prefetch_after_evict(i):
                nwb = 2 if wb_cursor[0] <= i + 3 else 1
                for _ in range(nwb):
                    if wb_cursor[0] < OC:
                        wb_dma(wb_cursor[0])
                        wb_cursor[0] += 1
                if w8_gate[0] and w8_cursor[0] < OC:
                    w8_dma(w8_cursor[0])
                    w8_cursor[0] += 1

            def xr_stream(win, chunks):
                # fp8 residual chunk in, DVE add x8+xr8 -> bf16 window of xb
                for a, b in chunks:
                    st = xrpool.tile(
                        [P, XR_CHUNK, T_FREE], f8, tag="xr", name=f"xr_{win}_{a}"
                    )
                    nc.sync.dma_start(
                        st[:, : b - a, :], xtr[win, :, a:b, :]
                    )
                    nc.vector.tensor_tensor(
                        xb_sb[:, win, a:b, :],
                        x8_sb[:, win, a:b, :],
                        st[:, : b - a, :],
                        mybir.AluOpType.add,
                    )

            # PE warmup: junk DR matmuls on a DMA'd zeros tile (DMA starts
            # ~2.5us in, vs ~8us for an engine memset) lift the HAM clock
            # gate while the first weight/x tiles land.
            warm_sb = xpool.tile([P, 2, T_FREE], f8)
            warm_ps = pspool.tile([P, T_FREE], f32, tag="ps", name="warm")
            nc.sync.dma_start(warm_sb[:], z0[:])
            for _ in range(WARM):
                nc.tensor.matmul(
                    warm_ps[:],
                    warm_sb[:, :, 0:P],
                    warm_sb[:],
                    start=True,
                    stop=True,
                    perf_mode=DR,
                )

            # Startup DMA: sync carries inputs in priority order; scalar
            # carries wb0/wb1/wb2 concurrently.
            w8_dma(0)
            nc.sync.dma_start(x8_sb[:, 0, : KO // 2, :], xt8[0, :, : KO // 2, :])
            for i in range(1, HOIST):
                w8_dma(i)
            nc.sync.dma_start(x8_sb[:, 0, KO // 2 :, :], xt8[0, :, KO // 2 :, :])
            nc.sync.dma_start(bias_sb[:], bs[:])
            wb_dma(0)
            wb_dma(1)
            first0 = _chunk_runs(ksb[perm[0]], 2, XR_CHUNK)
            cov0 = {k for a, b in first0 for k in range(a, b)}
            rest0 = _chunk_runs([k for k in range(KO) if k not in cov0], 0, XR_CHUNK)
            xr_stream(0, first0)
            nc.sync.dma_start(x8_sb[:, 1, : KO // 2, :], xt8[1, :, : KO // 2, :])
            xr_stream(0, rest0)
            nc.sync.dma_start(x8_sb[:, 1, KO // 2 :, :], xt8[1, :, KO // 2 :, :])
            xr_stream(1, first0)
            xr_stream(1, rest0)

            def dr_pairs(oc):
                ks = ks8[oc]
                return [(ks[2 * j], ks[2 * j + 1]) for j in range(ns[oc] // 2)]

            def dr_mm(i, psv, win, lo2, width, oc, j2, start, stop):
                k1, k2 = ks8[oc][2 * j2], ks8[oc][2 * j2 + 1]
                nc.tensor.matmul(
                    psv,
                    w8_tiles[i][:, 2 * j2 : 2 * j2 + 2, :],
                    x8_sb[:, win, k1 : k2 + 1 : (k2 - k1), lo2 : lo2 + width],
                    start=start,
                    stop=stop,
                    perf_mode=DR,
                )

            def bf_mms(i, psv, win, lo2, width, oc, start, stop, prime=False):
                wbt = wb_tiles.get(i)
                m = KO - ns[oc]
                for j in range(m):
                    k = ksb[oc][j]
                    if j == 0 and prime and width > 64:
                        # narrow first instruction: absorbs the fresh wb
                        # tile's semaphore wait (which breaks the PE's
                        # weight-load lookahead) at 64-wide cost
                        nc.tensor.matmul(
                            psv[:, 0:64],
                            wbt[:, 0, :],
                            xb_sb[:, win, k, lo2 : lo2 + 64],
                            start=start,
                            stop=False,
                        )
                        nc.tensor.matmul(
                            psv[:, 64:width],
                            wbt[:, 0, :],
                            xb_sb[:, win, k, lo2 + 64 : lo2 + width],
                            start=False,
                            stop=(stop and m == 1),
                        )
                        continue
                    nc.tensor.matmul(
                        psv,
                        wbt[:, j, :],
                        xb_sb[:, win, k, lo2 : lo2 + width],
                        start=(start and j == 0),
                        stop=(stop and j == m - 1),
                    )

            # Hoisted DR groups: out-tiles 0..HOIST-1, window 0, on PSUM
            # banks 1..7 (bank 0 is the warmup's, freed immediately).
            # Low-k pairs for all tiles first so the PE doesn't wait on the
            # second half of x8 window 0.
            ps_handles = {}
            for i in range(HOIST):
                ps_handles[(i, 0)] = pspool.tile(
                    [P, T_FREE], f32, tag="ps", name=f"ps_{i}_0"
                )
            started = set()
            for phase in (0, 1):
                for i in range(HOIST):
                    oc = perm[i]
                    for j2, (k1, k2) in enumerate(dr_pairs(oc)):
                        if (k2 < KO // 2) == (phase == 0):
                            first = i not in started
                            if first:
                                started.add(i)
                            dr_mm(
                                i,
                                ps_handles[(i, 0)][:],
                                0,
                                0,
                                T_FREE,
                                oc,
                                j2,
                                start=first,
                                stop=False,
                            )
            # one extra hoisted group on the warmup's freed bank: the first
            # out-tile's window-1 DRs (x8 win 1 lands early), bridging the
            # PE over the bf16-x reconstruction stream
            if ns[perm[0]] > 0:
                ps_handles[(0, 1)] = pspool.tile(
                    [P, T_FREE], f32, tag="ps", name="ps_0_1"
                )
                for j2 in range(ns[perm[0]] // 2):
                    dr_mm(
                        0,
                        ps_handles[(0, 1)][:],
                        1,
                        0,
                        T_FREE,
                        perm[0],
                        j2,
                        start=(j2 == 0),
                        stop=False,
                    )

            def evict(i, oc, psv, lo, width, split):
                o_sb = opool.tile([P, T_FREE], f32, tag="o", name=f"o_{i}_{lo}")
                nc.scalar.activation(
                    o_sb[:, 0:width],
                    psv,
                    mybir.ActivationFunctionType.Identity,
                    bias=bias_sb[:, oc : oc + 1],
                    scale=S_OUT,
                )
                if not split:
                    nc.scalar.dma_start(
                        yt[oc, :, lo : lo + width], o_sb[:, 0:width]
                    )
                else:
                    pq = P // 2
                    for q in range(2):
                        eng = nc.sync if q == 1 else nc.scalar
                        eng.dma_start(
                            yt[oc, q * pq : (q + 1) * pq, lo : lo + width],
                            o_sb[q * pq : (q + 1) * pq, 0:width],
                        )
                prefetch_after_evict(i)

            def process(i, win):
                oc = perm[i]
                last = i == OC - 1
                if last and win == 1:
                    windows = [(0, 256), (256, 256)]
                else:
                    windows = [(0, T_FREE)]
                for lo2, width in windows:
                    ps = ps_handles.pop((i, win), None)
                    hoisted = ps is not None
                    if not hoisted:
                        ps = pspool.tile(
                            [P, T_FREE], f32, tag="ps", name=f"ps_{i}_{win}_{lo2}"
                        )
                    psv = ps[:, 0:width]
                    n = ns[oc]
                    if hoisted:
                        # group already started with its DRs; close with bf16
                        bf_mms(
                            i,
                            psv,
                            win,
                            lo2,
                            width,
                            oc,
                            start=(n == 0),
                            stop=True,
                            prime=(win == 0),
                        )
                    else:
                        # bf16 first (cheap weight load at the window
                        # boundary), fp8 DR pairs last
                        bf_mms(
                            i,
                            psv,
                            win,
                            lo2,
                            width,
                            oc,
                            start=True,
                            stop=(n == 0),
                            prime=(win == 0),
                        )
                        for j2 in range(n // 2):
                            dr_mm(
                                i,
                                psv,
                                win,
                                lo2,
                                width,
                                oc,
                                j2,
                                start=False,
                                stop=(j2 == n // 2 - 1),
                            )
                    lo = win * T_FREE + lo2
                    evict(i, oc, psv, lo, width, split=last)

            # Phase 1: close the hoisted window-0 groups with bf16.
            for i in range(HOIST):
                process(i, 0)
            # Phase 2: the hoisted out-tiles' window 1 (w8 prefetch opens).
            w8_gate[0] = True
            for i in range(HOIST):
                process(i, 1)
            # Phase 3: remaining out-tiles, both windows.
            for i in range(HOIST, OC):
                process(i, 0)
                process(i, 1)

    nc.compile()
    return nc


def _pack_inputs(x, weight, bias, plan, x8, xr8):
    import ml_dtypes

    E4 = ml_dtypes.float8_e4m3
    BF = ml_dtypes.bfloat16

    w8 = (weight * SW).astype(E4).reshape(OC, P, KO, P)
    wb = (weight * SW).astype(BF).reshape(OC, P, KO, P)

    xt8 = np.ascontiguousarray(
        x8.reshape(N_CORES, NT, T_FREE, KO, P).transpose(0, 1, 4, 3, 2)
    )
    xtr = np.ascontiguousarray(
        xr8.reshape(N_CORES, NT, T_FREE, KO, P).transpose(0, 1, 4, 3, 2)
    )
    wp8 = np.ascontiguousarray(
        np.concatenate(
            [w8[oc][:, plan["ks8"][oc], :].transpose(2, 1, 0) for oc in range(OC)],
            axis=1,
        )
    )
    wpb = np.ascontiguousarray(
        np.concatenate(
            [wb[oc][:, plan["ksb"][oc], :].transpose(2, 1, 0) for oc in range(OC)],
            axis=1,
        )
    )
    bsr = np.ascontiguousarray(bias.reshape(OC, P).T)
    return xt8, xtr, wp8, wpb, bsr


def kernel(x, weight, bias):
    global LAST_EXEC_NS
    from concourse import bass_utils

    x = np.ascontiguousarray(x, dtype=np.float32)
    weight = np.ascontiguousarray(weight, dtype=np.float32)
    bias = np.ascontiguousarray(bias, dtype=np.float32)

    x8, xr8, x8d, xbd = _quant_arrays(x)

    if "nc" not in _cache:
        sel = _select_tiles(x, weight, x8d, xbd)
        plan = _make_plan(sel)
        _cache["plan"] = plan
        _cache["nc"] = _build_bass(plan)
    nc = _cache["nc"]
    plan = _cache["plan"]

    xt8, xtr, wp8, wpb, bsr = _pack_inputs(x, weight, bias, plan, x8, xr8)
    import ml_dtypes

    z0 = np.zeros((P, 2, T_FREE), dtype=ml_dtypes.float8_e4m3)

    in_maps = [
        {
            "xt8": xt8[c],
            "xtr": xtr[c],
            "wp8": wp8,
            "wpb": wpb,
            "bs": bsr,
            "z0": z0,
        }
        for c in range(N_CORES)
    ]

    trace = bool(int(os.environ.get("BSL_TRACE", "0")))
    kw = {}
    if os.environ.get("BSL_TMPDIR"):
        kw["tmpdir"] = os.environ["BSL_TMPDIR"]
    res = bass_utils.run_bass_kernel_spmd(
        nc,
        in_maps,
        core_ids=list(range(N_CORES)),
        trace=trace,
        **kw,
    )
    _cache["res"] = res
    LAST_EXEC_NS = res.exec_time_ns

    out = np.empty((TOK, D_OUT), dtype=np.float32)
    for c in range(N_CORES):
        yt_out = res.results[c]["yt"]
        out[c * T_PER_CORE : (c + 1) * T_PER_CORE] = (
            yt_out.transpose(2, 0, 1).reshape(T_PER_CORE, D_OUT)
        )
    return out
